# revision 66
# baseline (speedup 1.0000x reference)
"""AR(1) sequence generation kernel for Trainium2 (8 NeuronCores).

x_t = COEFF * x_{t-1} + STD * eps_t  with x_0 given; output [B, T] f32.

Algorithm: the recurrence is a causal convolution x = conv(E, k) with
k[d] = STD * COEFF**d, where E[:,0] = x0/STD and E[:,1:] = eps.  Since
COEFF**128 ~ 4e-13 (far below f32 ulp), a 128-wide output time-block m
only depends on time-blocks m and m-1:

    X[:, m*128:(m+1)*128] = E_m @ M0 + E_{m-1} @ M1

with constant 128x128 Toeplitz matrices M0[j,i] = k[i-j] (i>=j) and
M1[j,i] = k[128+i-j].  This removes the serial scan entirely.

Default mode "v4fzp" (int8 in / int8 out, ~47.8us HW exec; "p" =
10 throwaway PE-warmup matmuls on a memset scratch tile during the
idle load gate, completing the measured 583/427->216ns p-state ramp
before the real stream starts (~0.5-1us); vs ~61-64 for
the earlier fp16 "v2"; "f" = fine tail store groups [4,4,8,8,4,2,1,1],
"z" = last two stores on HWDGE, each worth ~0.5us in an interleaved
4-round A/B once the input stream is already drained).  The 2e-2 error gate leaves ~50x headroom over
fp16, which buys 1-byte I/O in both directions (half of v2's HBM
bytes) — measured rel err 1.283e-2, bit-identical to the host-side
simulation used to pick the quantizers:
- INPUT: eps is quantized to s8 (step 4.2/127, noise-shaped on host:
  the quantization error is spectrally shaped by (1 - 0.8 z^-1) so the
  AR kernel attenuates it ~1.7x; the x0 column gets its own step).
  The SWDGE DMA *casts s8->f16 in flight* (verified bit-exact on HW),
  so the matmul sees exact integer f16 operands and neither DVE nor
  ACT spends a cycle converting.
- All scales live in the f16 Toeplitz weights (per-output-column s8
  output scale sout_i = 126.8/(5.0*std_i), per-input-row quantizer
  step) with special weight sets for time blocks 0/1 where std_t still
  varies and the x0 row enters.
- OUTPUT: DVE/ACT evacuate PSUM f32 straight to s8 — the hardware
  f32->s8 cast is round-to-nearest-even + saturating (verified on HW),
  so the s8 quantization costs zero extra engine work and clipping at
  5 sigma is free.  Host decodes x = s8/sout (one vectorized multiply).

Schedule (kept from v2): whole input resident up front, ALL bulk data
on ONE gpsimd SWDGE queue in FIFO order (input first, stores behind),
stores in [4,4,8,8,4,2,2] time-block groups.  v4 trace anatomy
(48.6us): ~6.1us fixed prologue (runtime init sem ~3.2 + TENSOR_LOAD
~1.3 + ordering setup), first matmul at 12.3 (weights DMA completion
semaphore), PE saturated (measured 100.0% occupancy, zero stream gaps
>60ns) for 29.5us of f16 matmul streaming (the hard floor of this
algorithm: 2 passes of N=512 cycles per output chunk; matmul cost is
free-dim cycles per pass regardless of contraction size, so rank-1 /
carry-folding / K-truncation tricks cannot reduce it; int8 PE modes
are fenced off by the bass layer + Rust cost model, and fp8
DoubleRow for the M1 pass measures 1.38x SLOWER per matmul than f16
on HW despite the cost model's 0.5x — see modes "d"/"dw"), stores
drain to 45.0, ~3.6us drain/epilogue.  The v2 DMA_15 straggler tail is gone (s8 stores
shrink the SWDGE descriptor-ring pressure that slowed engine 15).

Things measured NOT to help (each costs ~1-7us, single-queue FIFO on
SWDGE is load-bearing): HWDGE for weights, lead-in, or ALL stores
(competes with the input stream at the engine round-robin), reordering
the gpsimd enqueue, extra lead-in splits (each gpsimd dma_start costs
~0.65us serialized Q7 emission), weight-major matmul ordering (PE is
stream-bound, LDWEIGHTS already hidden).  HWDGE for only the LAST two
stores (after input drains) does help — that plus the fine tail is the
"fz" default.

Fallbacks kept: "v2"/"v3", "fp16ioN"/"fp16pN", "bf16x2p"/"bf16x2",
"f32" (exact).

Sharding: pure data parallel over batch (8192 -> 8 x 1024), no
communication.
"""

import os

import numpy as np

COEFF = 0.8
STD = 0.1
B, T = 8192, 4096
NCORES = 8
BL = B // NCORES  # 1024 batch rows per core
P = 128           # partition dim / time block
NBLK = T // P     # 32 time blocks
NCHUNK = BL // 512  # 2 batch chunks of 512

# "v4" (int8 in/out, cast-DMA input, ~half the HBM bytes of v2),
# "v2" (fp16 io, whole-input-resident schedule), "fp16pN" (fp16
# operands, N time blocks packed per DMA group), "bf16x2p"/"bf16x2"
# (bf16 hi/lo split), "f32" (exact, 4x slower PE)
MM_DTYPE = os.environ.get("AR_MM_DTYPE", "v4fzp")

# v4 quantization parameters (see docstring; validated by host sim
# against the reference: rel err ~1.28e-2 vs the 2e-2 gate).
V4_IN_RANGE = 4.2    # eps quantized to s8 with step IN_RANGE/127
V4_ZMAX = 5.0        # output range = ZMAX * std_t (RNE + saturate)
NSB = NBLK // 2  # superblocks of 2 time blocks (bf16x2p packed layout)

_CACHE = {}


def _toeplitz():
    j = np.arange(P, dtype=np.float64)[:, None]
    i = np.arange(P, dtype=np.float64)[None, :]
    m0 = np.where(i >= j, STD * COEFF ** (i - j), 0.0).astype(np.float32)
    m1 = (STD * COEFF ** (P + i - j)).astype(np.float32)
    return m0, m1


def _split_bf16(a):
    import ml_dtypes

    bf16 = ml_dtypes.bfloat16
    hi = a.astype(bf16)
    lo = (a - hi.astype(np.float32)).astype(bf16)
    return hi, lo


def _build_nc_f32():
    import concourse.bacc as bacc
    import concourse.tile as tile
    import concourse.mybir as mybir

    f32 = mybir.dt.float32
    nc = bacc.Bacc("TRN2", target_bir_lowering=False, debug=False)
    ein = nc.dram_tensor("ein", [T, BL], f32, kind="ExternalInput")
    m0 = nc.dram_tensor("m0", [P, P], f32, kind="ExternalInput")
    m1 = nc.dram_tensor("m1", [P, P], f32, kind="ExternalInput")
    out = nc.dram_tensor("out", [T, BL], f32, kind="ExternalOutput")

    with tile.TileContext(nc) as tc:
        with (
            tc.tile_pool(name="consts", bufs=1) as consts,
            tc.tile_pool(name="einp", bufs=4) as einp,
            tc.tile_pool(name="stage", bufs=3) as stage,
            tc.tile_pool(name="psum", bufs=4, space="PSUM") as psump,
        ):
            m0t = consts.tile([P, P], f32, name="m0t")
            nc.sync.dma_start(m0t[:], m0[:, :])
            m1t = consts.tile([P, P], f32, name="m1t")
            nc.sync.dma_start(m1t[:], m1[:, :])

            einv = ein[:, :].rearrange("(n p) b -> n p b", p=P)
            outv = out[:, :].rearrange("(n p) b -> n p b", p=P)

            prev = None
            for m in range(NBLK):
                et = einp.tile([P, BL], f32, name="et")
                nc.sync.dma_start(et[:], einv[m])
                st = stage.tile([P, BL], f32, name="st")
                for c in range(NCHUNK):
                    ps = psump.tile([P, 512], f32, name="ps")
                    sl = slice(c * 512, (c + 1) * 512)
                    nc.tensor.matmul(
                        ps[:], m0t[:], et[:, sl],
                        start=True, stop=(prev is None),
                    )
                    if prev is not None:
                        nc.tensor.matmul(
                            ps[:], m1t[:], prev[:, sl],
                            start=False, stop=True,
                        )
                    nc.vector.tensor_copy(out=st[:, sl], in_=ps[:])
                nc.sync.dma_start(outv[m], st[:])
                prev = et
    nc.compile()
    return nc


def _build_nc_bf16x2():
    import concourse.bacc as bacc
    import concourse.tile as tile
    import concourse.mybir as mybir

    f32 = mybir.dt.float32
    bf16 = mybir.dt.bfloat16
    nc = bacc.Bacc("TRN2", target_bir_lowering=False, debug=False)
    ein_hi = nc.dram_tensor("ein_hi", [T, BL], bf16, kind="ExternalInput")
    ein_lo = nc.dram_tensor("ein_lo", [T, BL], bf16, kind="ExternalInput")
    consts_in = {}
    for name in ("m0h", "m0l", "m1h", "m1l"):
        consts_in[name] = nc.dram_tensor(name, [P, P], bf16, kind="ExternalInput")
    out = nc.dram_tensor("out", [T, BL], f32, kind="ExternalOutput")

    with tile.TileContext(nc) as tc:
        with (
            tc.tile_pool(name="consts", bufs=1) as consts,
            tc.tile_pool(name="einp", bufs=8) as einp,
            tc.tile_pool(name="stage", bufs=3) as stage,
            tc.tile_pool(name="psum", bufs=4, space="PSUM") as psump,
        ):
            ct = {}
            for name in ("m0h", "m0l", "m1h", "m1l"):
                ct[name] = consts.tile([P, P], bf16, name=name + "t")
                nc.sync.dma_start(ct[name][:], consts_in[name][:, :])

            ehv = ein_hi[:, :].rearrange("(n p) b -> n p b", p=P)
            elv = ein_lo[:, :].rearrange("(n p) b -> n p b", p=P)
            outv = out[:, :].rearrange("(n p) b -> n p b", p=P)

            prev = None
            for m in range(NBLK):
                eh = einp.tile([P, BL], bf16, name="eh", tag="eh")
                nc.sync.dma_start(eh[:], ehv[m])
                el = einp.tile([P, BL], bf16, name="el", tag="el")
                nc.sync.dma_start(el[:], elv[m])
                st = stage.tile([P, BL], f32, name="st")
                for c in range(NCHUNK):
                    sl = slice(c * 512, (c + 1) * 512)
                    ps = psump.tile([P, 512], f32, name="ps")
                    # (weights, moving) products accumulated in PSUM;
                    # ordered so equal stationary operands are adjacent.
                    prods = [
                        (ct["m0h"], eh),
                        (ct["m0h"], el),
                        (ct["m0l"], eh),
                    ]
                    if prev is not None:
                        ph, pl = prev
                        prods += [
                            (ct["m1h"], ph),
                            (ct["m1h"], pl),
                            (ct["m1l"], ph),
                        ]
                    last = len(prods) - 1
                    for idx, (w, mv) in enumerate(prods):
                        nc.tensor.matmul(
                            ps[:], w[:], mv[:, sl],
                            start=(idx == 0), stop=(idx == last),
                        )
                    nc.vector.tensor_copy(out=st[:, sl], in_=ps[:])
                nc.sync.dma_start(outv[m], st[:])
                prev = (eh, el)
    nc.compile()
    return nc


def _build_nc_bf16x2p():
    """Packed layout: input DRAM tensor [NSB*128, 4096] bf16 where the
    row (sb*128 + p) holds, contiguously: [hi(sub0) | lo(sub0) | hi(sub1)
    | lo(sub1)] each 1024 wide, with sub-block rows interleaved so every
    partition line is one contiguous 8KB DMA descriptor.  Output DRAM is
    [NSB*128, 2048] f32: row (sb*128+p) = [x^T row 256sb+p | row
    256sb+128+p] (8KB lines); the host unpacks."""
    import concourse.bacc as bacc
    import concourse.tile as tile
    import concourse.mybir as mybir

    f32 = mybir.dt.float32
    bf16 = mybir.dt.bfloat16
    nc = bacc.Bacc("TRN2", target_bir_lowering=False, debug=False)
    einp = nc.dram_tensor("einp", [NSB * P, 4096], bf16, kind="ExternalInput")
    consts_in = {}
    for name in ("m0h", "m0l", "m1h", "m1l"):
        consts_in[name] = nc.dram_tensor(name, [P, P], bf16, kind="ExternalInput")
    outp = nc.dram_tensor("outp", [NSB * P, 2048], f32, kind="ExternalOutput")

    with tile.TileContext(nc) as tc:
        with (
            tc.tile_pool(name="consts", bufs=1) as consts,
            tc.tile_pool(name="ein", bufs=3) as ein_pool,
            tc.tile_pool(name="stage", bufs=3) as stage,
            tc.tile_pool(name="psum", bufs=8, space="PSUM") as psump,
        ):
            ct = {}
            for name in ("m0h", "m0l", "m1h", "m1l"):
                ct[name] = consts.tile([P, P], bf16, name=name + "t")
                nc.gpsimd.dma_start(ct[name][:], consts_in[name][:, :])

            # [NSB, p, sub, hilo, 1024]
            einv = einp[:, :].rearrange(
                "(n p) (s h b) -> n p s h b", p=P, s=2, h=2
            )
            outv = outp[:, :].rearrange("(n p) (s b) -> n p s b", p=P, s=2)

            prev = None  # (tile, sub) holding time block m-1
            for sb in range(NSB):
                et = ein_pool.tile([P, 2, 2, 1024], bf16, name="et")
                nc.sync.dma_start(et[:], einv[sb])
                st = stage.tile([P, 2, 1024], f32, name="st")
                for sub in range(2):
                    eh = et[:, sub, 0, :]
                    el = et[:, sub, 1, :]
                    for c in range(NCHUNK):
                        sl = slice(c * 512, (c + 1) * 512)
                        ps = psump.tile([P, 512], f32, name="ps")
                        prods = [
                            (ct["m0h"], eh),
                            (ct["m0h"], el),
                            (ct["m0l"], eh),
                        ]
                        if prev is not None:
                            pt, psub = prev
                            prods += [
                                (ct["m1h"], pt[:, psub, 0, :]),
                                (ct["m1h"], pt[:, psub, 1, :]),
                                (ct["m1l"], pt[:, psub, 0, :]),
                            ]
                        last = len(prods) - 1
                        for idx, (w, mv) in enumerate(prods):
                            nc.tensor.matmul(
                                ps[:], w[:], mv[:, sl],
                                start=(idx == 0), stop=(idx == last),
                            )
                        nc.vector.tensor_copy(out=st[:, sub, sl], in_=ps[:])
                    prev = (et, sub)
                nc.scalar.dma_start(outv[sb], st[:])
    nc.compile()
    return nc


def _build_nc_v2(hybrid=False, fine_tail=False):
    """Whole-input-resident schedule, fp16 in/out (see module docstring).

    The full 8MB input shard is loaded up front (it fits in SBUF with a
    full set of stage tiles, so nothing ever waits on buffer rotation),
    and all bulk data travels one gpsimd/SWDGE queue in strict FIFO
    order: input first (compute finishes earliest), output stores
    behind it (engines never idle while work exists, and descriptor
    generation stays off the straggling DMA engine 15).  `hybrid` moves
    the consts + first 1MB of input to sync/HWDGE, whose issue is
    ~0.4us/DMA cheaper, waking the engines slightly earlier.
    """
    import concourse.bacc as bacc
    import concourse.tile as tile
    import concourse.mybir as mybir

    subs = 8
    ngrp = NBLK // subs  # 4
    f32 = mybir.dt.float32
    f16 = mybir.dt.float16
    nc = bacc.Bacc("TRN2", target_bir_lowering=False, debug=False)
    einp = nc.dram_tensor("einp", [ngrp * P, subs * BL], f16, kind="ExternalInput")
    # m0 | m1 side by side: one DMA with 512B lines instead of two with
    # 256B lines (descriptor generation runs on DMA engine 15, which is
    # the straggler - keep its packet count down)
    mm_in = nc.dram_tensor("mm", [P, 2 * P], f16, kind="ExternalInput")
    outp = nc.dram_tensor("outp", [ngrp * P, subs * BL], f16, kind="ExternalOutput")

    # output store groups, in time-block units.  A small first group
    # starts the output stream ~4us earlier (output flow is
    # cast-production-limited, so every early byte shipped cuts the
    # final drain); small tail groups keep the last store's enqueue
    # ahead of the DMA engines.
    # fine_tail splits the last 2-block store in two: when compute is
    # power-throttled the final store is enqueue-gated (production
    # limited), so enqueueing the last MB in halves lands the final
    # bytes earlier.
    store_sizes = [4, 4, 8, 8, 4, 2, 1, 1] if fine_tail else [4, 4, 8, 8, 4, 2, 2]

    with tile.TileContext(nc) as tc:
        with (
            tc.tile_pool(name="consts", bufs=1) as consts,
            tc.tile_pool(name="ein", bufs=ngrp) as ein_pool,
            tc.tile_pool(name="stage", bufs=len(store_sizes)) as stage,
            tc.tile_pool(name="psum", bufs=8, space="PSUM") as psump,
        ):
            # ALL data goes on the single gpsimd SWDGE queue, input first:
            # - SWDGE (gpsimd generates descriptors) keeps descriptor-gen
            #   duty off DMA engine 15, which otherwise runs ~1.7x slower
            #   and serializes the last couple MB alone for 5-8us.
            # - One queue = strict FIFO: input streams at the full ~410GB/s
            #   (compute finishes earliest), output drains behind it,
            #   engines never idle while any work exists.
            # hybrid=True routes the lead-in DMAs via sync/HWDGE for a
            # cheaper issue; measured MUCH slower end-to-end (72-76us vs
            # 55-63us, interleaved A/B) - engaging the HWDGE path at all
            # appears to disturb the engine schedule.  Default: gpsimd.
            lead = nc.sync if hybrid else nc.gpsimd
            mmt = consts.tile([P, 2 * P], f16, name="mmt")
            lead.dma_start(mmt[:], mm_in[:, :])
            m0t = mmt[:, 0:P]
            m1t = mmt[:, P:2 * P]

            einv = einp[:, :].rearrange("(n p) (s b) -> n p s b", p=P, s=subs)
            outv = outp[:, :].rearrange("(n p) (s b) -> n p s b", p=P, s=subs)

            # all input loads up front, no rotation deps (bufs == ngrp).
            # Group 0 arrives as 2+2+4 block pieces so the first matmul
            # can start ~5us earlier; the rest as full 2MB groups (16KB
            # lines, peak DMA rate) on the gpsimd SWDGE queue, where the
            # output stores follow in strict FIFO order.
            ets = []
            for g in range(ngrp):
                et = ein_pool.tile([P, subs, BL], f16, name="et")
                if g == 0:
                    lead.dma_start(et[:, 0:2], einv[g][:, 0:2])
                    lead.dma_start(et[:, 2:4], einv[g][:, 2:4])
                    lead.dma_start(et[:, 4:8], einv[g][:, 4:8])
                else:
                    nc.gpsimd.dma_start(et[:], einv[g])
                ets.append(et)

            # NOTE: only DVE (vector) and ACT (scalar) can read PSUM;
            # GPSIMD cannot (BIR verifier rejects it).
            cast_engines = [nc.vector, nc.scalar]
            sub_of = lambda m: (ets[m // subs], m % subs)

            m = 0  # global time-block index
            ci = 0  # cast round-robin index
            for si, ssz in enumerate(store_sizes):
                # uniform shape so the pool rotates one tile kind; only
                # the first ssz sub-blocks are written/stored
                st = stage.tile([P, subs, BL], f16, name="st")
                m_lo = m
                for k in range(ssz):
                    cur_t, cur_s = sub_of(m)
                    for c in range(NCHUNK):
                        sl = slice(c * 512, (c + 1) * 512)
                        ps = psump.tile([P, 512], f32, name="ps")
                        if m == 0:
                            nc.tensor.matmul(
                                ps[:], m0t, cur_t[:, cur_s, sl],
                                start=True, stop=True,
                            )
                        else:
                            prv_t, prv_s = sub_of(m - 1)
                            nc.tensor.matmul(
                                ps[:], m0t, cur_t[:, cur_s, sl],
                                start=True, stop=False,
                            )
                            nc.tensor.matmul(
                                ps[:], m1t, prv_t[:, prv_s, sl],
                                start=False, stop=True,
                            )
                        eng = cast_engines[ci % 2]
                        ci += 1
                        if eng is nc.scalar:
                            eng.copy(out=st[:, k, sl], in_=ps[:])
                        else:
                            eng.tensor_copy(out=st[:, k, sl], in_=ps[:])
                    m += 1
                # store this group: rows are contiguous (ssz*2KB) slices of
                # the 16KB-row packed layout.  Issued via gpsimd = SOFTWARE
                # descriptor generation: HWDGE runs on DMA engine 15, which
                # is the measured straggler (it serializes the final ~2MB
                # for ~8us after the other 15 engines go idle); SWDGE keeps
                # store descriptors off it entirely.
                g0 = m_lo // subs
                s0 = m_lo % subs
                nc.gpsimd.dma_start(outv[g0][:, s0:s0 + ssz], st[:, :ssz])
    nc.compile()
    return nc


V4_WSCALE = 0.85 / 126.0  # s8 weight-plane scale (max |W| ~0.82 at ZMAX=5)


V4_DR_FIRST = 8   # first block whose M1 pass runs fp8-DoubleRow
V4_DR_K = 32      # M1 contraction truncated to last K rows of prev block


def _build_nc_v4(store_sizes=None, hw_lead=False, wmajor=False,
                 in0_first=False, hw_stores=False, lead_split=False,
                 merged_lead=False, hw_tail_stores=0, combined_lead=False,
                 dr_m1=False, gate_fine=False, prewarm=0):
    """int8-I/O variant of the v2 schedule (~half the HBM bytes).

    Input einp is s8: eps quantized (noise-shaped on host) with step
    IN_RANGE/127; the x0 column (time 0) with its own step0.  The DMA
    loads CAST s8->f16 in flight (SWDGE cast, bit-exact: integers are
    f16-representable), so the matmul sees exact integer f16 operands
    and no engine cycles are spent converting.  All scales live in the
    f16 Toeplitz weights: W[j,i] *= instep_j * sout_i, where sout_i =
    126.8/(ZMAX*std_i) is the per-output-time s8 quantization scale
    (per-column for block 0 where std_t still varies, converged
    constant for t>=128).  PSUM f32 is evacuated by DVE/ACT directly
    to s8 (hardware cast = round-to-nearest-even + saturate, verified
    on HW), and stores write 1 byte/elem.  Host decodes x = s8/sout.

    Weight slots in the single [P, 4P] const tile:
      0: W0s  (block 0, M0, per-col sout, row 0 scaled by step0)
      1: W1b  (block 1, M1 operand=E_0, row 0 scaled by step0)
      2: Wma  (blocks >=1, M0, converged sout)
      3: Wmb  (blocks >=2, M1, converged sout)
    """
    import concourse.bacc as bacc
    import concourse.tile as tile
    import concourse.mybir as mybir

    subs = 8
    ngrp = NBLK // subs  # 4
    f32 = mybir.dt.float32
    f16 = mybir.dt.float16
    s8 = mybir.dt.int8
    nc = bacc.Bacc("TRN2", target_bir_lowering=False, debug=False)
    einp = nc.dram_tensor("einp", [ngrp * P, subs * BL], s8, kind="ExternalInput")
    if combined_lead:
        # [E block0 | E block1 | Wh plane | Wl plane] per partition row:
        # ONE cast-DMA delivers the first two time blocks AND the s8
        # hi/lo weight planes (W = wscale*(Wh + Wl/254), recombined to
        # f16 by DVE while the DMA stream continues) — one fewer
        # serialized ~0.65us Q7 emission and an earlier first matmul.
        lead_in = nc.dram_tensor("lead", [P, 2 * BL + 8 * P], s8,
                                 kind="ExternalInput")
    else:
        ww_in = nc.dram_tensor("ww", [P, 4 * P], f16, kind="ExternalInput")
    if dr_m1:
        # fp8 copy of the last V4_DR_K rows of blocks DR_FIRST-1..30
        # (the only rows the truncated M1 needs) + the fp8 M1 weights,
        # packed [16, 2, nslot, 1024] DoubleRow-interleaved: virtual
        # contraction row r = 2k + a.  16 partitions, one DMA.
        fp8 = mybir.dt.float8e4
        ndrs = NBLK - V4_DR_FIRST               # 24 moving slots
        e8_in = nc.dram_tensor(
            "einp8", [V4_DR_K // 2, 2 * (ndrs + 1) * BL], fp8,
            kind="ExternalInput",
        )
    if hw_lead:
        # first 2 time blocks duplicated as f16 so they can ride HWDGE
        # (RTL descriptor gen; the s8 cast path is SWDGE-only)
        e0_in = nc.dram_tensor("e0", [P, 2 * BL], f16, kind="ExternalInput")
    outp = nc.dram_tensor("outp", [ngrp * P, subs * BL], s8, kind="ExternalOutput")

    if store_sizes is None:
        store_sizes = [4, 4, 8, 8, 4, 2, 2]

    with tile.TileContext(nc) as tc:
        with (
            tc.tile_pool(name="consts", bufs=1) as consts,
            tc.tile_pool(name="ein", bufs=ngrp) as ein_pool,
            tc.tile_pool(name="stage", bufs=len(store_sizes)) as stage,
            tc.tile_pool(
                name="psum", bufs=(7 if prewarm else 8), space="PSUM"
            ) as psump,
            tc.tile_pool(name="psum2", bufs=1, space="PSUM") as psump2,
        ):
            lead = nc.sync if hw_lead else nc.gpsimd
            lt = None
            if combined_lead:
                lt = consts.tile([P, 2 * BL + 8 * P], f16, name="lt")
                nc.gpsimd.dma_start(lt[:], lead_in[:, :])
                wwt = consts.tile([P, 4 * P], f16, name="wwt")
                whv = lt[:, 2 * BL:2 * BL + 4 * P]
                wlv = lt[:, 2 * BL + 4 * P:2 * BL + 8 * P]
                tl = consts.tile([P, 4 * P], f32, name="tl")
                nc.vector.tensor_scalar_mul(tl[:], wlv, V4_WSCALE / 254.0)
                nc.vector.scalar_tensor_tensor(
                    out=wwt[:], in0=whv, scalar=float(V4_WSCALE),
                    in1=tl[:], op0=mybir.AluOpType.mult,
                    op1=mybir.AluOpType.add,
                )
            else:
                wwt = consts.tile([P, 4 * P], f16, name="wwt")
                if lead_split or merged_lead:
                    # w0s slice first (32KB): the first matmul only needs it
                    lead.dma_start(wwt[:, 0:P], ww_in[:, 0:P])
                elif not in0_first:
                    lead.dma_start(wwt[:], ww_in[:, :])
            if prewarm:
                # Throwaway matmuls on a memset scratch tile during the
                # otherwise-idle load gate: the PE p-state ramp (measured
                # 583/427/216ns per matmul over the first ~10) completes
                # before the first real matmul, which then streams at max
                # clock from slice one (~2.3us saved at the stream head).
                dsc = consts.tile([P, 512], f16, name="dsc")
                nc.vector.memset(dsc[:], 0.0)
                dps = psump2.tile([P, 512], f32, name="dps")
                for _ in range(prewarm):
                    nc.tensor.matmul(
                        dps[:], dsc[:, 0:P], dsc[:],
                        start=True, stop=True,
                    )
            w0s = wwt[:, 0:P]
            w1b = wwt[:, P:2 * P]
            wma = wwt[:, 2 * P:3 * P]
            wmb = wwt[:, 3 * P:4 * P]

            einv = einp[:, :].rearrange("(n p) (s b) -> n p s b", p=P, s=subs)
            outv = outp[:, :].rearrange("(n p) (s b) -> n p s b", p=P, s=subs)

            # all input loads up front via SWDGE cast-DMA (s8 dram ->
            # f16 sbuf); group 0 in 2+2+4 block pieces so the first
            # matmul starts early.  One gpsimd queue, input first then
            # stores: strict FIFO, input gets priority.
            ets = []
            for g in range(ngrp):
                et = ein_pool.tile([P, subs, BL], f16, name="et")
                if g == 0 and hw_lead:
                    # first blocks ride HWDGE with the weights: RTL
                    # descriptor gen, no Q7 emission wait
                    nc.sync.dma_start(
                        et[:, 0:2], e0_in[:, :].rearrange("p (s b) -> p s b", s=2)
                    )
                    nc.gpsimd.dma_start(et[:, 2:4], einv[g][:, 2:4])
                    nc.gpsimd.dma_start(et[:, 4:8], einv[g][:, 4:8])
                elif g == 0 and merged_lead:
                    nc.gpsimd.dma_start(et[:, 0:2], einv[g][:, 0:2])
                    nc.gpsimd.dma_start(wwt[:, P:4 * P], ww_in[:, P:4 * P])
                    nc.gpsimd.dma_start(et[:, 2:4], einv[g][:, 2:4])
                    nc.gpsimd.dma_start(et[:, 4:8], einv[g][:, 4:8])
                elif g == 0 and lead_split:
                    nc.gpsimd.dma_start(et[:, 0:1], einv[g][:, 0:1])
                    nc.gpsimd.dma_start(wwt[:, P:4 * P], ww_in[:, P:4 * P])
                    nc.gpsimd.dma_start(et[:, 1:2], einv[g][:, 1:2])
                    nc.gpsimd.dma_start(et[:, 2:4], einv[g][:, 2:4])
                    nc.gpsimd.dma_start(et[:, 4:8], einv[g][:, 4:8])
                elif g == 0 and combined_lead:
                    # blocks 0-1 arrive inside the lead tile
                    nc.gpsimd.dma_start(et[:, 2:4], einv[g][:, 2:4])
                    nc.gpsimd.dma_start(et[:, 4:8], einv[g][:, 4:8])
                elif g == 0 and gate_fine:
                    # finer group-0 pieces: each op's sem fires earlier
                    # (one sem per op), at the cost of 2 extra 0.65us Q7
                    # emissions that finish before the first store needs
                    # the queue
                    nc.gpsimd.dma_start(et[:, 0:2], einv[g][:, 0:2])
                    nc.gpsimd.dma_start(et[:, 2:4], einv[g][:, 2:4])
                    nc.gpsimd.dma_start(et[:, 4:6], einv[g][:, 4:6])
                    nc.gpsimd.dma_start(et[:, 6:7], einv[g][:, 6:7])
                    nc.gpsimd.dma_start(et[:, 7:8], einv[g][:, 7:8])
                elif g == 0:
                    nc.gpsimd.dma_start(et[:, 0:2], einv[g][:, 0:2])
                    if in0_first:
                        nc.gpsimd.dma_start(wwt[:], ww_in[:, :])
                    nc.gpsimd.dma_start(et[:, 2:4], einv[g][:, 2:4])
                    nc.gpsimd.dma_start(et[:, 4:8], einv[g][:, 4:8])
                    if dr_m1:
                        e8t = consts.tile(
                            [V4_DR_K // 2, 2, ndrs + 1, BL],
                            mybir.dt.float8e4, name="e8t",
                        )
                        nc.gpsimd.dma_start(
                            e8t[:],
                            e8_in[:, :].rearrange(
                                "k (a m b) -> k a m b", a=2, b=BL
                            ),
                        )
                else:
                    nc.gpsimd.dma_start(et[:], einv[g])
                ets.append(et)

            cast_engines = [nc.vector, nc.scalar]

            def moving(m, sl):
                """[P, 512] f16 moving operand for time block m, chunk sl."""
                if combined_lead and m < 2:
                    return lt[:, m * BL + sl.start:m * BL + sl.stop]
                return ets[m // subs][:, m % subs, sl]

            def evac(ci, ps, dst):
                eng = cast_engines[ci % 2]
                if eng is nc.scalar:
                    eng.copy(out=dst, in_=ps)
                else:
                    eng.tensor_copy(out=dst, in_=ps)

            m = 0
            ci = 0
            for si, ssz in enumerate(store_sizes):
                st = stage.tile([P, subs, BL], s8, name="st")
                m_lo = m
                if not wmajor:
                    for k in range(ssz):
                        for c in range(NCHUNK):
                            sl = slice(c * 512, (c + 1) * 512)
                            ps = psump.tile([P, 512], f32, name="ps")
                            if m == 0:
                                nc.tensor.matmul(
                                    ps[:], w0s, moving(m, sl),
                                    start=True, stop=True,
                                )
                            elif dr_m1 and m >= V4_DR_FIRST:
                                nc.tensor.matmul(
                                    ps[:], wma, moving(m, sl),
                                    start=True, stop=False,
                                )
                                nc.tensor.matmul(
                                    ps[:],
                                    e8t[:, :, ndrs, 0:P],
                                    e8t[:, :, m - V4_DR_FIRST, sl],
                                    start=False, stop=True,
                                    perf_mode=mybir.MatmulPerfMode.DoubleRow,
                                )
                            else:
                                nc.tensor.matmul(
                                    ps[:], wma, moving(m, sl),
                                    start=True, stop=False,
                                )
                                nc.tensor.matmul(
                                    ps[:], w1b if m == 1 else wmb,
                                    moving(m - 1, sl),
                                    start=False, stop=True,
                                )
                            evac(ci, ps[:], st[:, k, sl])
                            ci += 1
                        m += 1
                else:
                    # weight-major: within a sub-batch of <=4 blocks
                    # (<=8 PSUM banks), run all M0 matmuls under one
                    # LDWEIGHTS, then all M1 matmuls, then evacuate.
                    for b0 in range(0, ssz, 4):
                        nb = min(4, ssz - b0)
                        pss = []
                        for k in range(nb):
                            mm = m + k
                            for c in range(NCHUNK):
                                sl = slice(c * 512, (c + 1) * 512)
                                ps = psump.tile([P, 512], f32, name="ps")
                                pss.append(ps)
                                nc.tensor.matmul(
                                    ps[:],
                                    w0s if mm == 0 else wma,
                                    moving(mm, sl),
                                    start=True, stop=(mm == 0),
                                )
                        for k in range(nb):
                            mm = m + k
                            for c in range(NCHUNK):
                                sl = slice(c * 512, (c + 1) * 512)
                                ps = pss[k * NCHUNK + c]
                                if dr_m1 and mm >= V4_DR_FIRST:
                                    nc.tensor.matmul(
                                        ps[:],
                                        e8t[:, :, ndrs, 0:P],
                                        e8t[:, :, mm - V4_DR_FIRST, sl],
                                        start=False, stop=True,
                                        perf_mode=mybir.MatmulPerfMode.DoubleRow,
                                    )
                                elif mm > 0:
                                    nc.tensor.matmul(
                                        ps[:],
                                        w1b if mm == 1 else wmb,
                                        moving(mm - 1, sl),
                                        start=False, stop=True,
                                    )
                                evac(ci, ps[:], st[:, b0 + k, sl])
                                ci += 1
                        m += nb
                g0 = m_lo // subs
                s0 = m_lo % subs
                hw_this = hw_stores or (
                    hw_tail_stores and si >= len(store_sizes) - hw_tail_stores
                )
                store_eng = nc.sync if hw_this else nc.gpsimd
                store_eng.dma_start(outv[g0][:, s0:s0 + ssz], st[:, :ssz])
    nc.compile()
    return nc


def _v4_scales():
    """(std_t[:P], sout_blk0[P], sout_c) for the s8 output quantizer."""
    t = np.arange(P, dtype=np.float64)
    var_t = 0.64 ** t + 0.01 * (1.0 - 0.64 ** t) / 0.36
    std_t = np.sqrt(var_t)
    std_c = np.sqrt(0.01 / 0.36)
    sout_blk0 = 126.8 / (V4_ZMAX * std_t)
    sout_c = 126.8 / (V4_ZMAX * std_c)
    return std_t, sout_blk0, sout_c


def _v4_weights(step0):
    """[P, 4P] f16 weight tile: W0s | W1b | Wma | Wmb (see _build_nc_v4)."""
    m0, m1 = _toeplitz()
    m0 = m0.astype(np.float64)
    m1 = m1.astype(np.float64)
    step = V4_IN_RANGE / 127.0
    _, sout_blk0, sout_c = _v4_scales()
    instep = np.full(P, step)
    instep[0] = step0
    w0s = m0 * instep[:, None] * sout_blk0[None, :]
    w1b = m1 * instep[:, None] * sout_c
    wma = m0 * step * sout_c
    wmb = m1 * step * sout_c
    ww = np.concatenate([w0s, w1b, wma, wmb], axis=1).astype(np.float16)
    return np.ascontiguousarray(ww)


def _v4_quant_input(x0, eps):
    """Noise-shaped s8 quantization of the input.  Returns (E_s8 [T, B],
    step0).  Shaped feedback: e'_t = e_t + COEFF*delta_{t-1} pushes the
    quantization error spectrum to where the AR kernel attenuates it
    (~1.6x lower output error than plain rounding)."""
    step = V4_IN_RANGE / 127.0
    col0 = x0[:, 0].astype(np.float64) / STD
    step0 = float(np.abs(col0).max() / 126.5)
    if step0 == 0.0:
        step0 = 1.0
    q0 = np.rint(col0 / step0).astype(np.int8)

    try:
        import jax
        import jax.numpy as jnp
        from jax import lax

        def shape_quant(eps_f32):
            def f(delta, e):
                v = e + np.float32(COEFF) * delta
                q = jnp.clip(jnp.rint(v * np.float32(1.0 / step)), -127, 127)
                return v - q * np.float32(step), q.astype(jnp.int8)
            _, qs = lax.scan(
                f, jnp.zeros(eps_f32.shape[0], jnp.float32), eps_f32.T
            )
            return qs

        cpu = jax.devices("cpu")[0]
        with jax.default_device(cpu):
            qT = np.asarray(jax.jit(shape_quant)(jnp.asarray(eps)))  # [T-1, B]
    except Exception:
        # numpy fallback: plain (unshaped) rounding; still passes the
        # gate (sim: 1.45e-2 vs 1.28e-2 shaped)
        qT = np.clip(
            np.rint(eps.astype(np.float32) / step), -127, 127
        ).astype(np.int8).T

    E = np.empty((T, B), np.int8)
    E[0] = q0
    E[1:] = qT
    return E, step0


def _pack_input_s8(shard, subs=8):
    """shard: [T, BL] s8 -> [(NBLK/subs)*P, subs*BL] s8 packed."""
    ngrp = NBLK // subs
    h = shard.reshape(ngrp, subs, P, BL).transpose(0, 2, 1, 3)
    return np.ascontiguousarray(h.reshape(ngrp * P, subs * BL))


def _build_nc_fp16p(subs=2, ein_bufs=6, stage_bufs=4, out_f16=False):
    """Pure fp16 operands (rel err ~3e-4), packed DMA layout.

    Input DRAM [(NBLK/subs)*P, subs*1024] fp16: row (g*P+p) holds the
    `subs` E^T rows {subs*128*g + 128*s + p} concatenated, so every
    partition line is one contiguous (2KB*subs) DMA descriptor.  Output
    DRAM is packed the same way in f32 (4KB*subs lines) - or fp16 when
    out_f16 (halves output traffic, adds ~1e-4 quantization error; the
    host upcasts).  Two matmul products per output chunk."""
    import concourse.bacc as bacc
    import concourse.tile as tile
    import concourse.mybir as mybir

    ngrp = NBLK // subs
    f32 = mybir.dt.float32
    f16 = mybir.dt.float16
    out_dt = f16 if out_f16 else f32
    nc = bacc.Bacc("TRN2", target_bir_lowering=False, debug=False)
    einp = nc.dram_tensor("einp", [ngrp * P, subs * BL], f16, kind="ExternalInput")
    m0_in = nc.dram_tensor("m0", [P, P], f16, kind="ExternalInput")
    m1_in = nc.dram_tensor("m1", [P, P], f16, kind="ExternalInput")
    outp = nc.dram_tensor("outp", [ngrp * P, subs * BL], out_dt, kind="ExternalOutput")

    with tile.TileContext(nc) as tc:
        with (
            tc.tile_pool(name="consts", bufs=1) as consts,
            tc.tile_pool(name="ein", bufs=ein_bufs) as ein_pool,
            tc.tile_pool(name="stage", bufs=stage_bufs) as stage,
            tc.tile_pool(name="psum", bufs=8, space="PSUM") as psump,
        ):
            m0t = consts.tile([P, P], f16, name="m0t")
            nc.sync.dma_start(m0t[:], m0_in[:, :])
            m1t = consts.tile([P, P], f16, name="m1t")
            nc.sync.dma_start(m1t[:], m1_in[:, :])

            einv = einp[:, :].rearrange("(n p) (s b) -> n p s b", p=P, s=subs)
            outv = outp[:, :].rearrange("(n p) (s b) -> n p s b", p=P, s=subs)

            split0 = os.environ.get("AR_SPLIT0", "1") == "1"
            prev = None  # (tile, sub) holding time block m-1
            for g in range(ngrp):
                et = ein_pool.tile([P, subs, BL], f16, name="et")
                if g == 0 and split0 and subs >= 4:
                    q = subs // 4
                    for k in range(4):
                        nc.sync.dma_start(
                            et[:, k * q:(k + 1) * q],
                            einv[g][:, k * q:(k + 1) * q],
                        )
                else:
                    nc.sync.dma_start(et[:], einv[g])
                st = stage.tile([P, subs, BL], out_dt, name="st")
                for sub in range(subs):
                    for c in range(NCHUNK):
                        sl = slice(c * 512, (c + 1) * 512)
                        ps = psump.tile([P, 512], f32, name="ps")
                        if prev is None:
                            nc.tensor.matmul(
                                ps[:], m0t[:], et[:, sub, sl],
                                start=True, stop=True,
                            )
                        else:
                            pt, psub = prev
                            nc.tensor.matmul(
                                ps[:], m0t[:], et[:, sub, sl],
                                start=True, stop=False,
                            )
                            nc.tensor.matmul(
                                ps[:], m1t[:], pt[:, psub, sl],
                                start=False, stop=True,
                            )
                        if (sub + c) % 2 == 0:
                            nc.vector.tensor_copy(out=st[:, sub, sl], in_=ps[:])
                        else:
                            nc.scalar.copy(out=st[:, sub, sl], in_=ps[:])
                    prev = (et, sub)
                nc.scalar.dma_start(outv[g], st[:])
    nc.compile()
    return nc


def _get_nc(mm_dtype=MM_DTYPE):
    if mm_dtype not in _CACHE:
        if mm_dtype.startswith("v4") or mm_dtype == "v5":
            opts = mm_dtype[2:] if mm_dtype.startswith("v4") else "hfw"
            _CACHE[mm_dtype] = _build_nc_v4(
                store_sizes=(
                    [4, 4, 8, 8, 4, 2, 1, 1] if "f" in opts else None
                ),
                hw_lead=("h" in opts),
                wmajor=("w" in opts),
                in0_first=("i" in opts),
                hw_stores=("s" in opts),
                lead_split=("l" in opts),
                merged_lead=("m" in opts),
                hw_tail_stores=(3 if "3" in opts else (2 if "z" in opts else 0)),
                combined_lead=("c" in opts),
                dr_m1=("d" in opts),
                gate_fine=("g" in opts),
                prewarm=(10 if "p" in opts else 0),
            )
        elif mm_dtype in ("v2", "v2h", "v3"):
            _CACHE[mm_dtype] = _build_nc_v2(hybrid=(mm_dtype == "v2h"), fine_tail=(mm_dtype == "v3"))
        elif mm_dtype.startswith("fp16p") or mm_dtype.startswith("fp16io"):
            pfx = "fp16io" if mm_dtype.startswith("fp16io") else "fp16p"
            subs = int(mm_dtype[len(pfx):]) if len(mm_dtype) > len(pfx) else 2
            if subs >= 8:
                import os as _os
                _b = _os.environ.get("AR_BUFS")
                if _b:
                    e, s = (int(v) for v in _b.split(","))
                    kw = {"ein_bufs": e, "stage_bufs": s}
                else:
                    kw = ({"ein_bufs": 6, "stage_bufs": 4} if pfx == "fp16io"
                          else {"ein_bufs": 3, "stage_bufs": 3})
            else:
                kw = {}
            _CACHE[mm_dtype] = _build_nc_fp16p(
                subs=subs, out_f16=(pfx == "fp16io"), **kw
            )
        elif mm_dtype == "bf16x2p":
            _CACHE[mm_dtype] = _build_nc_bf16x2p()
        elif mm_dtype == "bf16x2":
            _CACHE[mm_dtype] = _build_nc_bf16x2()
        elif mm_dtype == "f32":
            _CACHE[mm_dtype] = _build_nc_f32()
        else:
            raise ValueError(mm_dtype)
    return _CACHE[mm_dtype]


def _make_e(x0, eps):
    e = np.empty((B, T), np.float32)
    e[:, 0] = (x0[:, 0].astype(np.float64) / STD).astype(np.float32)
    e[:, 1:] = eps
    return np.ascontiguousarray(e.T)  # [T, B]


def _pack_input(shard):
    """shard: [T, BL] f32 (x^T for one core) -> [NSB*P, 4096] bf16 packed."""
    hi, lo = _split_bf16(shard)
    # [NSB, sub, P, BL] -> [NSB, P, sub, BL]
    hi4 = hi.reshape(NSB, 2, P, BL).transpose(0, 2, 1, 3)
    lo4 = lo.reshape(NSB, 2, P, BL).transpose(0, 2, 1, 3)
    packed = np.empty((NSB, P, 2, 2, BL), hi.dtype)
    packed[:, :, :, 0, :] = hi4
    packed[:, :, :, 1, :] = lo4
    return np.ascontiguousarray(packed.reshape(NSB * P, 4 * BL))


def _unpack_output(arr, subs=2):
    """arr: [(NBLK/subs)*P, subs*BL] f32 -> [BL, T] (batch-major shard)."""
    ngrp = NBLK // subs
    a = arr.reshape(ngrp, P, subs, BL).transpose(0, 2, 1, 3).reshape(T, BL)
    return a.T


def _pack_input_fp16(shard, subs=2):
    """shard: [T, BL] f32 -> [(NBLK/subs)*P, subs*BL] fp16 packed."""
    ngrp = NBLK // subs
    h = shard.astype(np.float16).reshape(ngrp, subs, P, BL).transpose(0, 2, 1, 3)
    return np.ascontiguousarray(h.reshape(ngrp * P, subs * BL))


def _make_in_maps(x0, eps, mm_dtype=MM_DTYPE):
    if mm_dtype.startswith("v4") or mm_dtype == "v5":
        E, step0 = _v4_quant_input(np.asarray(x0), np.asarray(eps))
        ww = _v4_weights(step0)
        opts = mm_dtype[2:] if mm_dtype.startswith("v4") else "hfw"
        if "c" in opts:
            wf = ww.astype(np.float64)
            wh = np.clip(np.rint(wf / V4_WSCALE), -127, 127)
            wl = np.clip(
                np.rint((wf / V4_WSCALE - wh) * 254.0), -127, 127
            ).astype(np.int8)
            wh = wh.astype(np.int8)
            maps = []
            for c in range(NCORES):
                sh = slice(c * BL, (c + 1) * BL)
                lead = np.empty((P, 2 * BL + 8 * P), np.int8)
                lead[:, 0:BL] = E[0:P, sh]
                lead[:, BL:2 * BL] = E[P:2 * P, sh]
                lead[:, 2 * BL:2 * BL + 4 * P] = wh
                lead[:, 2 * BL + 4 * P:] = wl
                maps.append({
                    "einp": _pack_input_s8(E[:, sh]),
                    "lead": lead,
                })
            return maps
        e8_full = None
        wmb8 = None
        if "d" in opts:
            import concourse.mybir as _mybir

            np8 = _mybir.dt.np(_mybir.dt.float8e4)
            m0_, m1_ = _toeplitz()
            step = V4_IN_RANGE / 127.0
            _, _, sout_c = _v4_scales()
            kk = np.arange(V4_DR_K // 2)
            rows = P - V4_DR_K + (2 * kk[:, None] + np.arange(2)[None, :])
            wmb8 = (
                m1_.astype(np.float64)[rows.reshape(-1), :] * step * sout_c
            ).reshape(V4_DR_K // 2, 2, P).astype(np8)  # [16, 2, 128]
            ndrs = NBLK - V4_DR_FIRST
            gbs = V4_DR_FIRST - 1 + np.arange(ndrs)
            # E rows (gb*P + 96 + 2k + a) for all slots -> [16, 2, ndrs, B]
            ridx = (gbs[None, None, :] * P)[...] + rows[:, :, None]
            e8_full = E[ridx.reshape(-1), :].astype(np8).reshape(
                V4_DR_K // 2, 2, ndrs, B
            )
        maps = []
        for c in range(NCORES):
            m = {
                "einp": _pack_input_s8(E[:, c * BL:(c + 1) * BL]),
                "ww": ww,
            }
            if "d" in opts:
                ndrs = NBLK - V4_DR_FIRST
                arr = np.zeros(
                    (V4_DR_K // 2, 2, ndrs + 1, BL), wmb8.dtype
                )
                arr[:, :, :ndrs, :] = e8_full[:, :, :, c * BL:(c + 1) * BL]
                arr[:, :, ndrs, :P] = wmb8
                m["einp8"] = np.ascontiguousarray(
                    arr.reshape(V4_DR_K // 2, 2 * (ndrs + 1) * BL)
                )
            if "h" in opts:
                e0 = E[:2 * P, c * BL:(c + 1) * BL].astype(np.float16)
                m["e0"] = np.ascontiguousarray(
                    e0.reshape(2, P, BL).transpose(1, 0, 2).reshape(P, 2 * BL)
                )
            maps.append(m)
        return maps
    et = _make_e(x0, eps)
    m0, m1 = _toeplitz()
    if mm_dtype in ("v2", "v2h", "v3"):
        mmh = np.ascontiguousarray(
            np.concatenate([m0, m1], axis=1).astype(np.float16)
        )
        return [
            {
                "einp": _pack_input_fp16(et[:, c * BL:(c + 1) * BL], 8),
                "mm": mmh,
            }
            for c in range(NCORES)
        ]
    if mm_dtype.startswith("fp16p") or mm_dtype.startswith("fp16io"):
        pfx = "fp16io" if mm_dtype.startswith("fp16io") else "fp16p"
        subs = int(mm_dtype[len(pfx):]) if len(mm_dtype) > len(pfx) else 2
        m0h = m0.astype(np.float16)
        m1h = m1.astype(np.float16)
        return [
            {
                "einp": _pack_input_fp16(et[:, c * BL:(c + 1) * BL], subs),
                "m0": m0h,
                "m1": m1h,
            }
            for c in range(NCORES)
        ]
    if mm_dtype == "bf16x2p":
        m0h, m0l = _split_bf16(m0)
        m1h, m1l = _split_bf16(m1)
        return [
            {
                "einp": _pack_input(et[:, c * BL:(c + 1) * BL]),
                "m0h": m0h, "m0l": m0l, "m1h": m1h, "m1l": m1l,
            }
            for c in range(NCORES)
        ]
    if mm_dtype == "f32":
        return [
            {
                "ein": np.ascontiguousarray(et[:, c * BL:(c + 1) * BL]),
                "m0": m0,
                "m1": m1,
            }
            for c in range(NCORES)
        ]
    m0h, m0l = _split_bf16(m0)
    m1h, m1l = _split_bf16(m1)
    maps = []
    for c in range(NCORES):
        shard = et[:, c * BL:(c + 1) * BL]
        hi, lo = _split_bf16(shard)
        maps.append(
            {
                "ein_hi": np.ascontiguousarray(hi),
                "ein_lo": np.ascontiguousarray(lo),
                "m0h": m0h, "m0l": m0l, "m1h": m1h, "m1l": m1l,
            }
        )
    return maps


def _run(in_maps, mm_dtype=MM_DTYPE, **kwargs):
    from concourse.bass_utils import run_bass_kernel_spmd

    nc = _get_nc(mm_dtype)
    return run_bass_kernel_spmd(
        nc, in_maps, core_ids=list(range(NCORES)), **kwargs
    )


def _gather(res, mm_dtype=MM_DTYPE):
    out = np.empty((B, T), np.float32)
    if mm_dtype.startswith("v4") or mm_dtype == "v5":
        _, sout_blk0, sout_c = _v4_scales()
        sdec = np.empty(T, np.float32)
        sdec[:P] = (1.0 / sout_blk0).astype(np.float32)
        sdec[P:] = np.float32(1.0 / sout_c)
        for c in range(NCORES):
            arr = res.results[c]["outp"]  # s8 [ngrp*P, subs*BL]
            u = _unpack_output(arr.astype(np.float32), 8)  # [BL, T]
            out[c * BL:(c + 1) * BL, :] = u * sdec[None, :]
        return out
    for c in range(NCORES):
        if mm_dtype in ("v2", "v2h", "v3"):
            arr = res.results[c]["outp"].astype(np.float32)
            out[c * BL:(c + 1) * BL, :] = _unpack_output(arr, 8)
        elif mm_dtype.startswith("fp16p") or mm_dtype.startswith("fp16io"):
            pfx = "fp16io" if mm_dtype.startswith("fp16io") else "fp16p"
            subs = int(mm_dtype[len(pfx):]) if len(mm_dtype) > len(pfx) else 2
            arr = res.results[c]["outp"]
            if arr.dtype != np.float32:
                arr = arr.astype(np.float32)
            out[c * BL:(c + 1) * BL, :] = _unpack_output(arr, subs)
        elif mm_dtype == "bf16x2p":
            out[c * BL:(c + 1) * BL, :] = _unpack_output(res.results[c]["outp"])
        else:
            out[c * BL:(c + 1) * BL, :] = res.results[c]["out"].T
    return out


def kernel(x0, eps):
    in_maps = _make_in_maps(x0, eps)
    try:
        res = _run(in_maps)
    except Exception:
        # rare transient NRT execution failures have been observed on the
        # first run of a fresh NEFF; one retry has always succeeded
        import time

        time.sleep(5)
        res = _run(in_maps)
    return _gather(res)

